# revision 1
# baseline (speedup 1.0000x reference)
"""Trainium2 Bass kernel for nn_Decoder (2-layer LSTM decoder + vocab head).

Computation (matches reference.py):
  embeds = emb[sentence]                      [B, T, E]
  x = concat(features, embeds[:, :-1])        [B, T, E]
  h0 = LSTM0(x), h1 = LSTM1(h0)               [B, T, H]
  out = (h1 @ fc_W.T + fc_b).transpose(0,2,1) [B, V, T]

Sharding (8 NeuronCores, SPMD, no collectives): the LSTM is replicated
(sequential in T and weight-ingestion bound; AllGather-sharding the input
projections was implemented and measured SLOWER here -- the collectives
run far above their ~5us datasheet floor and the recurring stalls also
re-throttle the PE clock via HAM).  The fc/vocab dimension is sharded 8
ways: 4000 rows/core padded to 4096, written [V_loc, T, B], gathered and
transposed on the host.

Measured on trn2 (8 cores): ~471.7 us HW exec, rel err 1.34e-2 (vs the
483.8 us / 3.5e-3 all-bf16 predecessor).  PE matmul-busy is ~385 us of
the span: rec pairs ~33 ns (fp8 FWL), xp FD=256 ~120 ns, fc FD=512
~232 ns.

Design notes:
  - W_hh in fp8-e3m4, scaled x128 host-side (e3m4 min-normal 0.25; raw
    |w|<=0.044 would be subnormal), g-gate rows x2 more: FWL ingests 4
    fp8 weights/cycle vs 2 bf16, halving the FD=64 weight-load-bound
    recurrent LDWEIGHTS+MATMUL pair (~52ns -> ~30ns).  The moving operand
    (h history) stays bf16 - matmul operand dtypes may differ.  The scale
    rides the gate preactivation: the xp ring stores 32*(xp+b), the
    identity fold-in matrix is 4*I (exact in fp8), and the gate sigmoid
    applies scale=1/128.
  - All gates go through sigmoid: i,f,g,o stay in PyTorch order, g rows
    are pre-doubled, tanh(x)=2*sigmoid(2x)-1 is fixed up by one 4x-mode
    DVE tensor_scalar.  The sigmoid is split sigma(i,f,g) then sigma(o)
    so DVE starts the c-update while ACT computes o - shortening the
    serial gate chain measured ~10 us end-to-end.
  - xp0/xp1 are emitted as (4-slab, gate-chunk) units a few per
    recurrence step so xp matmuls fill the PE while ACT/DVE work the
    serial gate chain of the current step.
  - fc is t-major: unit (v-tile, 8-step block) reads the t-major h1
    history, so fc streams into the PE queue as soon as rec1 finishes
    each 8-step block and the 33.5 MB/core output DMA overlaps the LSTM.
  - Inputs load as full-tensor DMAs (16 KB per-partition-contiguous
    runs); column-sliced pieces emit 1 KB descriptors and measure slower.

Environment note: this walrus build rejects >1 embedded sync wait per
instruction; _split_waits_json() rewrites the serialized BIR, hoisting
excess waits onto same-engine NoOp carriers (identical semantics).
"""

import numpy as np
import ml_dtypes

# ---------------------------------------------------------------------------
# Workaround: this walrus build caps instructions at ONE embedded sync wait
# ("Too many sync wait commands" in setupSyncWait); Tile routinely attaches
# several.  Post-process the serialized BIR: hoist excess waits of every
# instruction onto same-engine NoOp carriers inserted immediately before it.
# ---------------------------------------------------------------------------
import orjson
import concourse.tile as tile

_MAXW = 1


def _split_waits_json(b: bytes) -> bytes:
    d = orjson.loads(b)
    for f in d["functions"]:
        for blk in f["blocks"]:
            out = []
            for inst in blk["instructions"]:
                si = inst.get("sync_info")
                if si:
                    w = si.get("on_wait") or []
                    if len(w) > _MAXW:
                        for i, wt in enumerate(w[:-_MAXW]):
                            out.append(
                                {
                                    "debug": inst.get("debug", 0),
                                    "engine": inst["engine"],
                                    "ins": [],
                                    "outs": [],
                                    "name": f"{inst['name']}-hw{i}",
                                    "opcode": "NoOp",
                                    "sync_info": {"on_update": [], "on_wait": [wt]},
                                }
                            )
                        si["on_wait"] = w[-_MAXW:]
                out.append(inst)
            blk["instructions"] = out
    return orjson.dumps(d)


def _patch_serialization(nc):
    orig = nc.to_json_bytes
    nc.to_json_bytes = lambda: _split_waits_json(orig())
    return nc


import concourse.bass as bass
import concourse.mybir as mybir
from concourse.bass import ts, ds
from concourse.bass_utils import run_bass_kernel_spmd

F32 = mybir.dt.float32
BF16 = mybir.dt.bfloat16
FP8E3 = mybir.dt.float8e3
AF = mybir.ActivationFunctionType
ALU = mybir.AluOpType
BF16_NP = ml_dtypes.bfloat16
FP8E3_NP = ml_dtypes.float8_e3m4

E, H, V, B, T = 512, 512, 32000, 64, 32
G = 4 * H                    # 2048 gate rows per layer
KC = 4                       # 512 = 4 k-chunks of 128
NCORES = 8
VPAD = 4096                  # per-core vocab slice, padded from 4000
NV = VPAD // 128             # 32 vocab tiles
NTOK = B * T                 # 2048
LAG = 6                      # rec1 runs LAG steps behind rec0
WSCALE = 128.0               # fp8 weight scale
RSCALE = 32.0                # xp-ring / gate-preactivation scale


def _build_nc():
    nc = bass.Bass()

    xT_d = nc.dram_tensor("xT", [128, KC, NTOK], BF16, kind="ExternalInput")
    wih0_d = nc.dram_tensor("wih0T", [128, KC, G], BF16, kind="ExternalInput")
    whh0_d = nc.dram_tensor("whh0T", [128, KC, G], FP8E3, kind="ExternalInput")
    wih1_d = nc.dram_tensor("wih1T", [128, KC, G], BF16, kind="ExternalInput")
    whh1_d = nc.dram_tensor("whh1T", [128, KC, G], FP8E3, kind="ExternalInput")
    b0_d = nc.dram_tensor("b0", [128, 16], F32, kind="ExternalInput")
    b1_d = nc.dram_tensor("b1", [128, 16], F32, kind="ExternalInput")
    ident_d = nc.dram_tensor("ident", [128, 128], FP8E3, kind="ExternalInput")
    fcw_d = nc.dram_tensor("fcwT", [128, KC, VPAD], BF16, kind="ExternalInput")
    fcb_d = nc.dram_tensor("fcb", [128, NV], F32, kind="ExternalInput")
    out_d = nc.dram_tensor("out", [VPAD, T, B], F32, kind="ExternalOutput")

    with tile.TileContext(nc) as tc:
        with (
            tc.tile_pool(name="consts", bufs=1) as consts,
            tc.tile_pool(name="state", bufs=1) as state,
            tc.tile_pool(name="fcpool", bufs=1) as fcpool,
            tc.tile_pool(name="fcstage", bufs=3) as fcstage,
            tc.tile_pool(name="ps_gates", bufs=2, space="PSUM") as ps_gates,
            tc.tile_pool(name="ps_big", bufs=4, space="PSUM") as ps_big,
        ):
            b0_sb = consts.tile([128, 16], F32, tag="b0")
            b1_sb = consts.tile([128, 16], F32, tag="b1")
            fcb_sb = consts.tile([128, NV], F32, tag="fcb")
            ident = consts.tile([128, 128], FP8E3, tag="ident")

            hist0 = consts.tile([128, KC, T, B], BF16, tag="hist0")   # t-major
            hist1 = consts.tile([128, KC, T, B], BF16, tag="hist1")   # t-major
            xp0r = consts.tile([128, 12, 16, B], BF16, tag="xp0r")
            xp1r = consts.tile([128, 8, 16, B], BF16, tag="xp1r")

            fcw_sb = fcpool.tile([128, KC, VPAD], BF16, tag="fcw")

            st = []
            for l in range(2):
                cT = state.tile([128, KC, B], F32, tag=f"cT{l}", name=f"cT{l}")
                gates = state.tile([128, 16, B], BF16, tag=f"gates{l}")
                g2 = state.tile([128, KC, B], BF16, tag=f"g2{l}")
                tmp1 = state.tile([128, KC, B], F32, tag=f"tmp1{l}")
                tmp2 = state.tile([128, KC, B], F32, tag=f"tmp2{l}")
                tanh_c = state.tile([128, KC, B], F32, tag=f"tanhc{l}")
                st.append(dict(cT=cT, gates=gates, g2=g2, tmp1=tmp1, tmp2=tmp2,
                               tanh_c=tanh_c))

            def xp_unit(w_sb, rhs_slice, bias_sb, ring, s0, nslab, g):
                """One (slab-range, gate-chunk) unit of an input projection."""
                n0, ntoks = s0 * B, nslab * B
                r0 = s0 % ring.shape[1]
                ps = ps_big.tile([128, 8, B], F32, tag="ps512")
                for kc in range(KC):
                    nc.tensor.matmul(
                        ps[:, 0:nslab, :],
                        w_sb[:, kc, ts(g, 128)],
                        rhs_slice(kc, n0, ntoks),
                        start=(kc == 0),
                        stop=(kc == KC - 1),
                    )
                dst = ring[:, ds(r0, nslab), g, :]
                if g % 2 == 0:
                    nc.scalar.activation(
                        out=dst, in_=ps[:, 0:nslab, :], func=AF.Identity,
                        bias=bias_sb[:, g : g + 1], scale=RSCALE,
                    )
                else:
                    nc.vector.tensor_scalar(
                        out=dst, in0=ps[:, 0:nslab, :],
                        scalar1=RSCALE, scalar2=bias_sb[:, g : g + 1],
                        op0=ALU.mult, op1=ALU.add,
                    )

            def fc_unit(u):
                """fc for vocab tile v = u%NV, time block tb = u//NV (8 steps)."""
                tb, v = u // NV, u % NV
                ps = ps_big.tile([128, 8, B], F32, tag="ps512")
                for kc in range(KC):
                    nc.tensor.matmul(
                        ps,
                        fcw_sb[:, kc, ts(v, 128)],
                        hist1[:, kc, ts(tb, 8), :],
                        start=(kc == 0),
                        stop=(kc == KC - 1),
                    )
                ot = fcstage.tile([128, 8, B], F32, tag="ot")
                if u % 2 == 0:
                    nc.scalar.activation(
                        out=ot, in_=ps, func=AF.Identity,
                        bias=fcb_sb[:, v : v + 1], scale=1.0,
                    )
                else:
                    nc.vector.tensor_scalar_add(ot, ps, fcb_sb[:, v : v + 1])
                eng = (nc.sync, nc.scalar)[u % 2]
                eng.dma_start(out=out_d[ts(v, 128), ts(tb, 8), :], in_=ot)

            def rec_step(l, t, whh_sb, ring, hist):
                s = st[l]
                xsl = ring[:, t % ring.shape[1], :, :]
                ps = ps_gates.tile([128, 16, B], F32, tag="psg")
                for half in (0, 1):
                    if t > 0:
                        for j in range(8):
                            gc = half * 8 + j
                            for kc in range(KC):
                                nc.tensor.matmul(
                                    ps[:, gc, :],
                                    whh_sb[:, kc, ts(gc, 128)],
                                    hist[:, kc, t - 1, :],
                                    start=(j == 0 and kc == 0),
                                    stop=False,
                                    skip_group_check=True,
                                )
                    nc.tensor.matmul(
                        ps[:, ts(half, 8), :],
                        ident,
                        xsl[:, ts(half, 8), :],
                        start=(t == 0),
                        stop=True,
                        skip_group_check=True,
                    )
                g = s["gates"]
                # i,f,g first so DVE starts the c-update while ACT does o
                nc.scalar.activation(g[:, 0:12, :], ps[:, 0:12, :],
                                     func=AF.Sigmoid, scale=1.0 / WSCALE)
                nc.scalar.activation(g[:, 12:16, :], ps[:, 12:16, :],
                                     func=AF.Sigmoid, scale=1.0 / WSCALE)
                nc.vector.tensor_scalar(
                    out=s["g2"], in0=g[:, 8:12, :],
                    scalar1=2.0, scalar2=1.0, op0=ALU.mult, op1=ALU.subtract,
                )
                if t == 0:
                    nc.vector.tensor_mul(s["cT"], g[:, 0:4, :], s["g2"])
                else:
                    nc.vector.tensor_mul(s["tmp1"], g[:, 0:4, :], s["g2"])
                    nc.vector.tensor_mul(s["tmp2"], g[:, 4:8, :], s["cT"])
                    nc.vector.tensor_add(s["cT"], s["tmp1"], s["tmp2"])
                nc.scalar.activation(s["tanh_c"], s["cT"], func=AF.Tanh)
                nc.vector.tensor_mul(hist[:, :, t, :], g[:, 12:16, :], s["tanh_c"])

            # fc emission bookkeeping: units become ready 32 per finished
            # 8-step block of rec1; drain a few per step to fill PE gaps.
            fc_state = {"done": 0, "ready": 0}

            def fc_ready(s_done):
                fc_state["ready"] = NV * ((s_done + 1) // 8)

            def fc_emit(k):
                n = min(fc_state["done"] + k, fc_state["ready"])
                for u in range(fc_state["done"], n):
                    fc_unit(u)
                fc_state["done"] = n

            with tc.tile_pool(name="wpool", bufs=1) as wpool:
                whh0_sb = wpool.tile([128, KC, G], FP8E3, tag="whh0")
                wih1_sb = wpool.tile([128, KC, G], BF16, tag="wih1")
                whh1_sb = wpool.tile([128, KC, G], FP8E3, tag="whh1")

                with tc.tile_pool(name="inpool", bufs=1) as inpool:
                    xT_sb = inpool.tile([128, KC, NTOK], BF16, tag="xT")
                    wih0_sb = inpool.tile([128, KC, G], BF16, tag="wih0")
                    # full-tensor loads: per-partition-contiguous 16 KB runs
                    # (column-sliced pieces would emit 1 KB descriptors and
                    # serialize the ring for ~35 us)
                    nc.scalar.dma_start(out=b0_sb, in_=b0_d[:])
                    nc.scalar.dma_start(out=b1_sb, in_=b1_d[:])
                    nc.scalar.dma_start(out=wih0_sb, in_=wih0_d[:])
                    nc.sync.dma_start(out=xT_sb, in_=xT_d[:])
                    nc.gpsimd.dma_start(out=whh0_sb, in_=whh0_d[:])
                    nc.scalar.dma_start(out=ident, in_=ident_d[:])
                    nc.scalar.dma_start(out=fcb_sb, in_=fcb_d[:])

                    xp0_rhs = lambda kc, n0, nt: xT_sb[:, kc, ds(n0, nt)]
                    xp1_rhs = lambda kc, n0, nt: hist0[:, kc, ds(n0 // B, nt // B), :]

                    def xp0_unit(s0, nslab, g):
                        xp_unit(wih0_sb, xp0_rhs, b0_sb, xp0r, s0, nslab, g)

                    def xp1_unit(s0, g):
                        xp_unit(wih1_sb, xp1_rhs, b1_sb, xp1r, s0, 4, g)

                    # slabs [0:8) up front (2 groups of 4); groups [8+4k:12+4k)
                    # spread 4 (group, g) units per step over t=4k..4k+3.
                    for g in range(16):
                        xp0_unit(0, 4, g)
                    for g in range(16):
                        xp0_unit(4, 4, g)
                    for t in range(24):
                        if t == 0:
                            nc.gpsimd.dma_start(out=wih1_sb, in_=wih1_d[:])
                        if t == 1:
                            nc.gpsimd.dma_start(out=whh1_sb, in_=whh1_d[:])
                        if t == 2:
                            nc.gpsimd.dma_start(out=fcw_sb, in_=fcw_d[:])
                        rec_step(0, t, whh0_sb, xp0r, hist0)
                        for gg in range(4):
                            xp0_unit(8 + 4 * (t // 4), 4, 4 * (t % 4) + gg)
                        if t % 4 == 3:
                            for g in range(8):
                                xp1_unit(t - 3, g)
                        elif t % 4 == 0 and t > 0:
                            for g in range(8, 16):
                                xp1_unit(t - 4, g)
                        if t >= LAG:
                            rec_step(1, t - LAG, whh1_sb, xp1r, hist1)
                            fc_ready(t - LAG)
                            fc_emit(4)
                for t in range(24, T):
                    rec_step(0, t, whh0_sb, xp0r, hist0)
                    if t % 4 == 3:
                        for g in range(8):
                            xp1_unit(t - 3, g)
                    elif t % 4 == 0:
                        for g in range(8, 16):
                            xp1_unit(t - 4, g)
                    rec_step(1, t - LAG, whh1_sb, xp1r, hist1)
                    fc_ready(t - LAG)
                    fc_emit(4)
                for g in range(8, 16):
                    xp1_unit(28, g)
                for s_ in range(T - LAG, T):
                    rec_step(1, s_, whh1_sb, xp1r, hist1)
                    fc_ready(s_)
                    fc_emit(5)
            fc_emit(4 * NV)
    return _patch_serialization(nc)


def _to_k128(W, dtype):
    """W [out_dim, K] -> [128, K//128, out_dim] with result[p,kc,g]=W[g,kc*128+p]."""
    K = W.shape[1]
    return np.ascontiguousarray(
        W.T.reshape(K // 128, 128, -1).transpose(1, 0, 2)
    ).astype(dtype)


_NC_CACHE = None
RUN_KWARGS = {}
LAST_RESULT = None


def kernel(
    sentence,
    features,
    lengths,
    emb,
    W_ih0,
    W_hh0,
    b_ih0,
    b_hh0,
    W_ih1,
    W_hh1,
    b_ih1,
    b_hh1,
    fc_W,
    fc_b,
):
    global _NC_CACHE, LAST_RESULT
    sentence = np.asarray(sentence).astype(np.int64)
    features = np.asarray(features, dtype=np.float32)
    emb = np.asarray(emb, dtype=np.float32)

    # embedding gather + teacher forcing shift (host; pure data movement)
    embeds = emb[sentence[:, : T - 1]]                      # [B, T-1, E]
    x = np.concatenate([features[:, None, :], embeds], axis=1)  # [B, T, E]
    xT = np.ascontiguousarray(x.transpose(2, 1, 0).reshape(E, NTOK))
    xT_p = np.ascontiguousarray(
        xT.reshape(KC, 128, NTOK).transpose(1, 0, 2)
    ).astype(BF16_NP)

    def prep_layer(W_ih, W_hh, b_ih, b_hh):
        """g-gate rows x2 (tanh->2*sigmoid-1), W_hh scaled into fp8-e3m4."""
        wih = np.asarray(W_ih, np.float32).copy()
        whh = np.asarray(W_hh, np.float32).copy()
        b = (np.asarray(b_ih, np.float32) + np.asarray(b_hh, np.float32)).copy()
        wih[2 * H : 3 * H] *= 2.0
        whh[2 * H : 3 * H] *= 2.0
        b[2 * H : 3 * H] *= 2.0
        wih_p = _to_k128(wih, BF16_NP)
        whh_p = _to_k128(whh * WSCALE, FP8E3_NP)
        b_p = np.ascontiguousarray((b * RSCALE).reshape(16, 128).T)
        return wih_p, whh_p, b_p

    wih0, whh0, b0 = prep_layer(W_ih0, W_hh0, b_ih0, b_hh0)
    wih1, whh1, b1 = prep_layer(W_ih1, W_hh1, b_ih1, b_hh1)

    fc_W = np.asarray(fc_W, np.float32)
    fc_b = np.asarray(fc_b, np.float32)
    vloc = V // NCORES  # 4000 real rows per core, padded to VPAD

    common = {
        "xT": xT_p,
        "wih0T": wih0,
        "whh0T": whh0,
        "wih1T": wih1,
        "whh1T": whh1,
        "b0": b0,
        "b1": b1,
        "ident": (np.eye(128, dtype=np.float32) * (WSCALE / RSCALE)).astype(FP8E3_NP),
    }
    in_maps = []
    for c in range(NCORES):
        wslice = np.zeros((VPAD, E), np.float32)
        wslice[:vloc] = fc_W[c * vloc : (c + 1) * vloc]
        bslice = np.zeros(VPAD, np.float32)
        bslice[:vloc] = fc_b[c * vloc : (c + 1) * vloc]
        wc = _to_k128(wslice, BF16_NP)
        bc = np.ascontiguousarray(bslice.reshape(NV, 128).T)
        in_maps.append({**common, "fcwT": wc, "fcb": bc})

    if _NC_CACHE is None:
        _NC_CACHE = _build_nc()

    res = run_bass_kernel_spmd(
        _NC_CACHE, in_maps, core_ids=list(range(NCORES)), **RUN_KWARGS
    )
    LAST_RESULT = res
    full = np.concatenate(
        [res.results[c]["out"][:vloc] for c in range(NCORES)], axis=0
    )  # [V, T, B]
    return np.ascontiguousarray(full.transpose(2, 0, 1))



# revision 6
# speedup vs baseline: 1.1171x; 1.1171x over previous
"""Trainium2 Bass kernel for nn_Decoder (2-layer LSTM decoder + vocab head).

Computation (matches reference.py):
  embeds = emb[sentence]                      [B, T, E]
  x = concat(features, embeds[:, :-1])        [B, T, E]
  h0 = LSTM0(x), h1 = LSTM1(h0)               [B, T, H]
  out = (h1 @ fc_W.T + fc_b).transpose(0,2,1) [B, V, T]

Sharding (8 NeuronCores, SPMD):
  - LSTM batch-sharded: each core runs xp+recurrence for its 8 of 64
    batches.  The recurrent matmuls are weight-load-bound, so shrinking
    the moving dim from 64 to 8 roughly halves their cost while the
    input projections (xp0/xp1, previously ~123us of replicated PE
    work) shrink 8x.
  - h1 is AllGathered per 8-step time block (64KB in -> 512KB out,
    bf16, HBM bounce buffers) so the vocab-sharded fc can consume all
    64 batches.
  - fc/vocab sharded 8 ways as before: 4000 rows/core padded to 4096,
    output written [V_loc, tb, core, t, b], reassembled on host.

Numerics identical to the 471.7us replicated baseline (rel err
~1.34e-2): W_hh in fp8-e3m4 scaled x128, g-gate rows x2
(tanh(x)=2*sigmoid(2x)-1), xp staged as bf16 32*(xp+b), identity
fold-in matrix 4*I in e3m4, gate sigmoid scale 1/128.

Environment note: this walrus build rejects >1 embedded sync wait per
instruction; _split_waits_json() rewrites the serialized BIR, hoisting
excess waits onto same-engine NoOp carriers (identical semantics).
"""

import numpy as np
import ml_dtypes

import orjson
import concourse.tile as tile

_MAXW = 1


def _split_waits_json(b: bytes) -> bytes:
    d = orjson.loads(b)
    for f in d["functions"]:
        for blk in f["blocks"]:
            out = []
            for inst in blk["instructions"]:
                si = inst.get("sync_info")
                if si:
                    w = si.get("on_wait") or []
                    if len(w) > _MAXW:
                        for i, wt in enumerate(w[:-_MAXW]):
                            out.append(
                                {
                                    "debug": inst.get("debug", 0),
                                    "engine": inst["engine"],
                                    "ins": [],
                                    "outs": [],
                                    "name": f"{inst['name']}-hw{i}",
                                    "opcode": "NoOp",
                                    "sync_info": {"on_update": [], "on_wait": [wt]},
                                }
                            )
                        si["on_wait"] = w[-_MAXW:]
                out.append(inst)
            blk["instructions"] = out
    return orjson.dumps(d)


def _patch_serialization(nc):
    orig = nc.to_json_bytes
    nc.to_json_bytes = lambda: _split_waits_json(orig())
    return nc


import concourse.bass as bass
import concourse.mybir as mybir
from concourse.bass import ts, ds
from concourse.bass_utils import run_bass_kernel_spmd

F32 = mybir.dt.float32
BF16 = mybir.dt.bfloat16
FP8E3 = mybir.dt.float8e3
AF = mybir.ActivationFunctionType
ALU = mybir.AluOpType
BF16_NP = ml_dtypes.bfloat16
FP8E3_NP = ml_dtypes.float8_e3m4

E, H, V, B, T = 512, 512, 32000, 64, 32
G = 4 * H                    # 2048 gate rows per layer
KC = 4                       # 512 = 4 k-chunks of 128
NCORES = 8
BL = B // NCORES             # 8 local batches per core
NTOKL = T * BL               # 256 local tokens
VPAD = 4096                  # per-core vocab slice, padded from 4000
NV = VPAD // 128             # 32 vocab tiles
NTB = 4                      # time blocks of 8 steps
TBS = T // NTB               # 8 steps per block
LAG = 9                      # rec1 runs LAG steps behind rec0
AG_DELAY = 7                 # rec1 steps from AG issue to fc-unit readiness
WSCALE = 128.0               # fp8 weight scale
RSCALE = 32.0                # xp / gate-preactivation scale
GROUPS = [list(range(NCORES))]


def _build_nc():
    nc = bass.Bass()

    xT_d = nc.dram_tensor("xT", [128, KC, NTOKL], BF16, kind="ExternalInput")
    wih0_d = nc.dram_tensor("wih0T", [128, KC, G], BF16, kind="ExternalInput")
    whh0_d = nc.dram_tensor("whh0T", [128, KC, G], FP8E3, kind="ExternalInput")
    wih1_d = nc.dram_tensor("wih1T", [128, KC, G], BF16, kind="ExternalInput")
    whh1_d = nc.dram_tensor("whh1T", [128, KC, G], FP8E3, kind="ExternalInput")
    b0_d = nc.dram_tensor("b0", [128, 16], F32, kind="ExternalInput")
    b1_d = nc.dram_tensor("b1", [128, 16], F32, kind="ExternalInput")
    ident_d = nc.dram_tensor("ident", [128, 128], FP8E3, kind="ExternalInput")
    fcw_d = nc.dram_tensor("fcwT", [128, KC, VPAD], BF16, kind="ExternalInput")
    fcb_d = nc.dram_tensor("fcb", [128, NV], F32, kind="ExternalInput")
    # out[v, tb, core, t, b]
    out_d = nc.dram_tensor("out", [VPAD, NTB, NCORES, TBS, BL], F32,
                           kind="ExternalOutput")

    with tile.TileContext(nc) as tc:
        with (
            tc.tile_pool(name="consts", bufs=1) as consts,
            tc.tile_pool(name="state", bufs=1) as state,
            tc.tile_pool(name="fcpool", bufs=1) as fcpool,
            tc.tile_pool(name="fcstage", bufs=3) as fcstage,
            tc.tile_pool(name="agdram", bufs=1, space="DRAM") as agdram,
            tc.tile_pool(name="ps_gates", bufs=2, space="PSUM") as ps_gates,
            tc.tile_pool(name="ps_xp", bufs=2, space="PSUM") as ps_xp,
            tc.tile_pool(name="ps_fc", bufs=3, space="PSUM") as ps_fc,
        ):
            b0_sb = consts.tile([128, 16], F32, tag="b0")
            b1_sb = consts.tile([128, 16], F32, tag="b1")
            fcb_sb = consts.tile([128, NV], F32, tag="fcb")
            ident = consts.tile([128, 128], FP8E3, tag="ident")

            # local-batch LSTM state, t-major [p, t, kc, b]
            hist0 = consts.tile([128, T, KC, BL], BF16, tag="hist0")
            hist1 = consts.tile([128, T, KC, BL], BF16, tag="hist1")
            # staged input projections, [p, gate, tok] (t-major tokens)
            xp0s = consts.tile([128, 16, NTOKL], BF16, tag="xp0s")
            xp1s = consts.tile([128, 16, NTOKL], BF16, tag="xp1s")
            # gathered h1: [p, tb, core, t, kc, b] (t-major per core chunk,
            # matching the bounce layout so the gather-back DMA is contiguous)
            hist1g = consts.tile([128, NTB, NCORES, TBS, KC, BL], BF16,
                                 tag="hist1g")

            fcw_sb = fcpool.tile([128, KC, VPAD], BF16, tag="fcw")

            agin = [agdram.tile([128, KC * TBS * BL], BF16, tag=f"agin{i}",
                                name=f"agin{i}")
                    for i in range(NTB)]
            agout = [agdram.tile([NCORES * 128, KC * TBS * BL], BF16,
                                 tag=f"agout{i}", name=f"agout{i}")
                     for i in range(NTB)]

            st = []
            for l in range(2):
                cT = state.tile([128, KC, BL], F32, tag=f"cT{l}", name=f"cT{l}")
                gates = state.tile([128, 16, BL], BF16, tag=f"gates{l}")
                g2 = state.tile([128, KC, BL], BF16, tag=f"g2{l}")
                tmp1 = state.tile([128, KC, BL], F32, tag=f"tmp1{l}")
                tmp2 = state.tile([128, KC, BL], F32, tag=f"tmp2{l}")
                tanh_c = state.tile([128, KC, BL], F32, tag=f"tanhc{l}")
                st.append(dict(cT=cT, gates=gates, g2=g2, tmp1=tmp1, tmp2=tmp2,
                               tanh_c=tanh_c))

            def xp0_unit(w_sb, g):
                """Full input projection for gate-chunk g over all 256 local
                tokens."""
                ps = ps_xp.tile([128, NTOKL], F32, tag="psxp")
                for kc in range(KC):
                    nc.tensor.matmul(
                        ps,
                        w_sb[:, kc, ts(g, 128)],
                        xT_sb[:, kc, :],
                        start=(kc == 0),
                        stop=(kc == KC - 1),
                    )
                dst = xp0s[:, g, :]
                if g % 2 == 0:
                    nc.scalar.activation(
                        out=dst, in_=ps, func=AF.Identity,
                        bias=b0_sb[:, g : g + 1], scale=RSCALE,
                    )
                else:
                    nc.vector.tensor_scalar(
                        out=dst, in0=ps,
                        scalar1=RSCALE, scalar2=b0_sb[:, g : g + 1],
                        op0=ALU.mult, op1=ALU.add,
                    )

            def xp1_chunk(cs, g):
                """Project hist0 steps [8cs, 8cs+8) for gate-chunk g."""
                psf = ps_xp.tile([128, NTOKL], F32, tag="psxp")
                ps = psf[:, 0 : TBS * BL]
                for kc in range(KC):
                    nc.tensor.matmul(
                        ps,
                        wih1_sb[:, kc, ts(g, 128)],
                        hist0[:, ts(cs, TBS), kc, :],
                        start=(kc == 0),
                        stop=(kc == KC - 1),
                    )
                dst = xp1s[:, g, ds(cs * TBS * BL, TBS * BL)]
                if g % 2 == 0:
                    nc.scalar.activation(
                        out=dst, in_=ps, func=AF.Identity,
                        bias=b1_sb[:, g : g + 1], scale=RSCALE,
                    )
                else:
                    nc.vector.tensor_scalar(
                        out=dst, in0=ps,
                        scalar1=RSCALE, scalar2=b1_sb[:, g : g + 1],
                        op0=ALU.mult, op1=ALU.add,
                    )

            def fc_unit(u):
                """fc for vocab tile v = u%NV, time block tb = u//NV."""
                tb, v = u // NV, u % NV
                ps = ps_fc.tile([128, NCORES * TBS * BL], F32, tag="psfc")
                for kc in range(KC):
                    nc.tensor.matmul(
                        ps,
                        fcw_sb[:, kc, ts(v, 128)],
                        hist1g[:, tb, :, :, kc, :],
                        start=(kc == 0),
                        stop=(kc == KC - 1),
                    )
                ot = fcstage.tile([128, NCORES * TBS * BL], F32, tag="ot")
                if u % 2 == 0:
                    nc.scalar.activation(
                        out=ot, in_=ps, func=AF.Identity,
                        bias=fcb_sb[:, v : v + 1], scale=1.0,
                    )
                else:
                    nc.vector.tensor_scalar_add(ot, ps, fcb_sb[:, v : v + 1])
                eng = (nc.sync, nc.scalar)[u % 2]
                eng.dma_start(out=out_d[ts(v, 128), tb, :, :, :], in_=ot)

            def rec_step(l, t, whh_sb, xps, hist):
                s = st[l]
                ps = ps_gates.tile([128, 16, BL], F32, tag="psg")
                for half in (0, 1):
                    if t > 0:
                        for j in range(8):
                            gc = half * 8 + j
                            for kc in range(KC):
                                nc.tensor.matmul(
                                    ps[:, gc, :],
                                    whh_sb[:, kc, ts(gc, 128)],
                                    hist[:, t - 1, kc, :],
                                    start=(j == 0 and kc == 0),
                                    stop=False,
                                    skip_group_check=True,
                                )
                    # xsl: [p, 8 gates, BL] slice of staged xp (stride NTOKL
                    # over the gate dim, BL contiguous at token offset t*BL)
                    nc.tensor.matmul(
                        ps[:, ts(half, 8), :],
                        ident,
                        xps[:, ts(half, 8), ds(t * BL, BL)],
                        start=(t == 0),
                        stop=True,
                        skip_group_check=True,
                    )
                g = s["gates"]
                # i,f,g first so DVE starts the c-update while ACT does o
                nc.scalar.activation(g[:, 0:12, :], ps[:, 0:12, :],
                                     func=AF.Sigmoid, scale=1.0 / WSCALE)
                nc.scalar.activation(g[:, 12:16, :], ps[:, 12:16, :],
                                     func=AF.Sigmoid, scale=1.0 / WSCALE)
                nc.vector.tensor_scalar(
                    out=s["g2"], in0=g[:, 8:12, :],
                    scalar1=2.0, scalar2=1.0, op0=ALU.mult, op1=ALU.subtract,
                )
                if t == 0:
                    nc.vector.tensor_mul(s["cT"], g[:, 0:4, :], s["g2"])
                else:
                    nc.vector.tensor_mul(s["tmp1"], g[:, 0:4, :], s["g2"])
                    nc.vector.tensor_mul(s["tmp2"], g[:, 4:8, :], s["cT"])
                    nc.vector.tensor_add(s["cT"], s["tmp1"], s["tmp2"])
                nc.scalar.activation(s["tanh_c"], s["cT"], func=AF.Tanh)
                nc.vector.tensor_mul(hist[:, t, :, :], g[:, 12:16, :],
                                     s["tanh_c"])

            def ag_block(tb):
                """AllGather this core's h1 for time block tb across cores."""
                nc.sync.dma_start(out=agin[tb][:, :],
                                  in_=hist1[:, ts(tb, TBS), :, :])
                nc.gpsimd.collective_compute(
                    "AllGather",
                    ALU.bypass,
                    replica_groups=GROUPS,
                    ins=[agin[tb][:, :]],
                    outs=[agout[tb][:, :]],
                )
                for c in range(NCORES):
                    nc.gpsimd.dma_start(
                        out=hist1g[:, tb, c, :, :, :],
                        in_=agout[tb][ts(c, 128), :],
                    )

            # fc emission bookkeeping: block tb's 32 units become ready
            # AG_DELAY rec1-steps after its AllGather is issued.
            fc_state = {"done": 0, "ready": 0, "issued": {}}

            def fc_ready(s_done):
                for tb, s_issue in fc_state["issued"].items():
                    if s_done >= s_issue + AG_DELAY:
                        fc_state["ready"] = max(fc_state["ready"],
                                                NV * (tb + 1))

            def fc_emit(k):
                n = min(fc_state["done"] + k, fc_state["ready"])
                for u in range(fc_state["done"], n):
                    fc_unit(u)
                fc_state["done"] = n

            with tc.tile_pool(name="wpool", bufs=1) as wpool:
                whh0_sb = wpool.tile([128, KC, G], FP8E3, tag="whh0")
                wih1_sb = wpool.tile([128, KC, G], BF16, tag="wih1")
                whh1_sb = wpool.tile([128, KC, G], FP8E3, tag="whh1")

                with tc.tile_pool(name="inpool", bufs=1) as inpool:
                    xT_sb = inpool.tile([128, KC, NTOKL], BF16, tag="xT")
                    wih0_sb = inpool.tile([128, KC, G], BF16, tag="wih0")
                    nc.scalar.dma_start(out=b0_sb, in_=b0_d[:])
                    nc.scalar.dma_start(out=b1_sb, in_=b1_d[:])
                    nc.sync.dma_start(out=xT_sb, in_=xT_d[:])
                    nc.scalar.dma_start(out=wih0_sb, in_=wih0_d[:])
                    nc.gpsimd.dma_start(out=whh0_sb, in_=whh0_d[:])
                    nc.scalar.dma_start(out=ident, in_=ident_d[:])
                    nc.scalar.dma_start(out=fcb_sb, in_=fcb_d[:])

                    # input projection layer 0 (all tokens), then recurrence
                    for g in range(16):
                        xp0_unit(wih0_sb, g)
                        if g == 0:
                            nc.gpsimd.dma_start(out=wih1_sb, in_=wih1_d[:])
                        if g == 2:
                            nc.gpsimd.dma_start(out=whh1_sb, in_=whh1_d[:])
                        if g == 4:
                            nc.gpsimd.dma_start(out=fcw_sb, in_=fcw_d[:])

                    for t in range(T):
                        rec_step(0, t, whh0_sb, xp0s, hist0)
                        if t % TBS == TBS - 1:
                            for g in range(16):
                                xp1_chunk(t // TBS, g)
                        if t >= LAG:
                            s_ = t - LAG
                            rec_step(1, s_, whh1_sb, xp1s, hist1)
                            if s_ % TBS == TBS - 1:
                                ag_block(s_ // TBS)
                                fc_state["issued"][s_ // TBS] = s_
                            fc_ready(s_)
                            fc_emit(4)
                for s_ in range(T - LAG, T):
                    rec_step(1, s_, whh1_sb, xp1s, hist1)
                    if s_ % TBS == TBS - 1:
                        ag_block(s_ // TBS)
                        fc_state["issued"][s_ // TBS] = s_
                    fc_ready(s_ + 2)
                    fc_emit(6)
            fc_state["ready"] = NTB * NV
            fc_emit(NTB * NV)
    return _patch_serialization(nc)


def _to_k128(W, dtype):
    """W [out_dim, K] -> [128, K//128, out_dim] with result[p,kc,g]=W[g,kc*128+p]."""
    K = W.shape[1]
    return np.ascontiguousarray(
        W.T.reshape(K // 128, 128, -1).transpose(1, 0, 2)
    ).astype(dtype)


_NC_CACHE = None
RUN_KWARGS = {}
LAST_RESULT = None


def kernel(
    sentence,
    features,
    lengths,
    emb,
    W_ih0,
    W_hh0,
    b_ih0,
    b_hh0,
    W_ih1,
    W_hh1,
    b_ih1,
    b_hh1,
    fc_W,
    fc_b,
):
    global _NC_CACHE, LAST_RESULT
    sentence = np.asarray(sentence).astype(np.int64)
    features = np.asarray(features, dtype=np.float32)
    emb = np.asarray(emb, dtype=np.float32)

    # embedding gather + teacher forcing shift (host; pure data movement)
    embeds = emb[sentence[:, : T - 1]]                      # [B, T-1, E]
    x = np.concatenate([features[:, None, :], embeds], axis=1)  # [B, T, E]

    def prep_layer(W_ih, W_hh, b_ih, b_hh):
        """g-gate rows x2 (tanh->2*sigmoid-1), W_hh scaled into fp8-e3m4."""
        wih = np.asarray(W_ih, np.float32).copy()
        whh = np.asarray(W_hh, np.float32).copy()
        b = (np.asarray(b_ih, np.float32) + np.asarray(b_hh, np.float32)).copy()
        wih[2 * H : 3 * H] *= 2.0
        whh[2 * H : 3 * H] *= 2.0
        b[2 * H : 3 * H] *= 2.0
        wih_p = _to_k128(wih, BF16_NP)
        whh_p = _to_k128(whh * WSCALE, FP8E3_NP)
        b_p = np.ascontiguousarray((b * RSCALE).reshape(16, 128).T)
        return wih_p, whh_p, b_p

    wih0, whh0, b0 = prep_layer(W_ih0, W_hh0, b_ih0, b_hh0)
    wih1, whh1, b1 = prep_layer(W_ih1, W_hh1, b_ih1, b_hh1)

    fc_W = np.asarray(fc_W, np.float32)
    fc_b = np.asarray(fc_b, np.float32)
    vloc = V // NCORES  # 4000 real rows per core, padded to VPAD

    common = {
        "wih0T": wih0,
        "whh0T": whh0,
        "wih1T": wih1,
        "whh1T": whh1,
        "b0": b0,
        "b1": b1,
        "ident": (np.eye(128, dtype=np.float32) * (WSCALE / RSCALE)).astype(FP8E3_NP),
    }
    in_maps = []
    for c in range(NCORES):
        # this core's batch slice of the LSTM input, t-major [E, T, BL]
        xc = np.ascontiguousarray(
            x[c * BL : (c + 1) * BL].transpose(2, 1, 0).reshape(E, NTOKL)
        )
        xc_p = np.ascontiguousarray(
            xc.reshape(KC, 128, NTOKL).transpose(1, 0, 2)
        ).astype(BF16_NP)
        wslice = np.zeros((VPAD, E), np.float32)
        wslice[:vloc] = fc_W[c * vloc : (c + 1) * vloc]
        bslice = np.zeros(VPAD, np.float32)
        bslice[:vloc] = fc_b[c * vloc : (c + 1) * vloc]
        wc = _to_k128(wslice, BF16_NP)
        bc = np.ascontiguousarray(bslice.reshape(NV, 128).T)
        in_maps.append({**common, "xT": xc_p, "fcwT": wc, "fcb": bc})

    if _NC_CACHE is None:
        _NC_CACHE = _build_nc()

    res = run_bass_kernel_spmd(
        _NC_CACHE, in_maps, core_ids=list(range(NCORES)), **RUN_KWARGS
    )
    LAST_RESULT = res
    full = np.empty((B, V, T), np.float32)
    for c in range(NCORES):
        a = res.results[c]["out"][:vloc]          # [vloc, tb, core, t, b]
        a = a.transpose(2, 4, 0, 1, 3)            # [core, b, vloc, tb, t]
        full[:, c * vloc : (c + 1) * vloc, :] = a.reshape(B, vloc, T)
    return full
